# revision 18
# baseline (speedup 1.0000x reference)
"""Trainium2 Bass kernel for nn_AttentionLayer_9792525435162 (gnn_message_passing).

Math (reference reformulated, validated in fp64/fp32/fp16 numpy sims):
  hc    = normalize(h.reshape(N,8,16), axis=2)
  att   = softmax_k( <hc_k, G hc_k> )          with G = query @ key_w.T
  diagv = <nb_sum_k, G hc_k>                    with nb_sum = sum_m h[neighbors]
  med17 = clamp(hc, S7, S8)  where S7,S8 = central pair of the 16 gathered
          neighbor values per (node, feature)   [median-of-17 identity]
  middle= med17 / max(||sub||_2 over 17, 1e-12)
  c_f   = sum_k diagv_k (hc_kf - middle_kf)^2   (cov, k-independent)
  flag  = any_f(0 <= c_f < 1)                   (dets==NaN path in reference)
  3x routing iterations with p = flag ? 1e-6 : right/max(||right||,1e-12),
  right = exp(-0.5 sum_f c_f (ix_f - hc_kf)^2); x = normalize(ix).

Sharding: data-parallel over nodes, 8192 nodes/core on 8 cores. The h table
is replicated per-core (hi|lo fp16 split rows, 512B each) and gathered with
per-partition indirect DMAs. All rsqrts via exp(-0.5 ln(max(s,eps))) + one
Newton step so ACT stays on the natural_log_exp_and_others table set.
"""

import sys

if "/opt/trn_rl_repo" not in sys.path:
    sys.path.insert(0, "/opt/trn_rl_repo")

import numpy as np

import concourse.bacc as bacc
import concourse.tile as tile
from concourse import mybir
from concourse.bass import IndirectOffsetOnAxis
from concourse.bass_utils import run_bass_kernel_spmd

f32 = mybir.dt.float32
f16 = mybir.dt.float16
i32 = mybir.dt.int32

N, M, D, K, DD = 65536, 16, 128, 8, 16
NC = 8
NPC = N // NC          # nodes per core
P = 128
NT = NPC // P          # tiles per core (64)
G = 4                  # tiles per group
NGRP = NT // G

AL = mybir.AluOpType
AF = mybir.ActivationFunctionType

# Median-of-16 central-pair selection network (pruned Batcher odd-even
# mergesort; 53 CEs, 92 ops). (i, j, need_min, need_max).
MEDIAN_NET = [
    (0, 1, 1, 1), (2, 3, 1, 1), (0, 2, 1, 1), (1, 3, 1, 1), (1, 2, 1, 1),
    (4, 5, 1, 1), (6, 7, 1, 1), (4, 6, 1, 1), (5, 7, 1, 1), (5, 6, 1, 1),
    (0, 4, 1, 1), (2, 6, 1, 1), (2, 4, 1, 1), (1, 5, 1, 1), (3, 7, 1, 1),
    (3, 5, 1, 1), (1, 2, 1, 1), (3, 4, 1, 1), (5, 6, 1, 1), (8, 9, 1, 1),
    (10, 11, 1, 1), (8, 10, 1, 1), (9, 11, 1, 1), (9, 10, 1, 1),
    (12, 13, 1, 1), (14, 15, 1, 1), (12, 14, 1, 1), (13, 15, 1, 1),
    (13, 14, 1, 1), (8, 12, 1, 1), (10, 14, 1, 1), (10, 12, 1, 1),
    (9, 13, 1, 1), (11, 15, 1, 1), (11, 13, 1, 1), (9, 10, 1, 1),
    (11, 12, 1, 1), (13, 14, 1, 1), (0, 8, 0, 1), (4, 12, 1, 0),
    (4, 8, 0, 1), (2, 10, 0, 1), (6, 14, 1, 0), (6, 10, 1, 0), (6, 8, 0, 1),
    (1, 9, 0, 1), (5, 13, 1, 0), (5, 9, 0, 1), (3, 11, 0, 1), (7, 15, 1, 0),
    (7, 11, 1, 0), (7, 9, 1, 0), (7, 8, 1, 1),
]


def _rsqrt(nc, sb, tag, src_ap, shape, newton_src=None):
    """out = 1/sqrt(src) via exp(-0.5*ln(src)) + one Newton step.

    src must already be clamped positive. newton_src: AP of the clamped s
    (defaults to src_ap). Returns the refined [*, ...] f32 tile.
    """
    ln_t = sb.tile(shape, f32, tag=tag + "_ln")
    nc.scalar.activation(out=ln_t[:], in_=src_ap, func=AF.Ln)
    y0 = sb.tile(shape, f32, tag=tag + "_y0")
    nc.scalar.activation(out=y0[:], in_=ln_t[:], func=AF.Exp, scale=-0.5)
    if newton_src is None:
        newton_src = src_ap
    # y1 = y0 * (1.5 - 0.5*s*y0^2)
    t1 = sb.tile(shape, f32, tag=tag + "_t1")
    nc.vector.tensor_tensor(out=t1[:], in0=y0[:], in1=y0[:], op=AL.mult)
    nc.vector.tensor_tensor(out=t1[:], in0=t1[:], in1=newton_src, op=AL.mult)
    nc.vector.tensor_scalar(out=t1[:], in0=t1[:], scalar1=-0.5, scalar2=1.5,
                            op0=AL.mult, op1=AL.add)
    nc.vector.tensor_tensor(out=t1[:], in0=t1[:], in1=y0[:], op=AL.mult)
    return t1


def build_program(iterat: int, debug=False):
    nc = bacc.Bacc("TRN2", target_bir_lowering=False)

    htab = nc.dram_tensor("htab", [N, 256], f16, kind="ExternalInput")
    hself = nc.dram_tensor("hself", [NPC, D], f32, kind="ExternalInput")
    nbr = nc.dram_tensor("nbr", [NPC, M], i32, kind="ExternalInput")
    wg = nc.dram_tensor("wg", [D, D], f32, kind="ExternalInput")  # blockdiag(G.T)
    out = nc.dram_tensor("out", [NPC, 24], f32, kind="ExternalOutput")
    dbg = {}
    _dbg_all = [("hc", 128), ("nbs", 128), ("ad", 16), ("med", 128),
                ("mid", 128), ("cc", 16), ("flag", 1), ("ssq", 128),
                ("ix0", 16)]
    if debug:
        sel_dbg = [x for x in _dbg_all
                   if debug is True or x[0] in debug]
        for nm, w in sel_dbg:
            dbg[nm] = nc.dram_tensor("dbg_" + nm, [NPC, w], f32,
                                     kind="ExternalOutput")
        dbg_v = {nm: t[:, :].rearrange("(t p) c -> p t c", p=P)
                 for nm, t in dbg.items()}

    idf_c = nc.dram_tensor("idf_c", [128, 128], f32, kind="ExternalInput")
    idh_c = nc.dram_tensor("idh_c", [128, 128], f16, kind="ExternalInput")
    sela_c = nc.dram_tensor("sela_c", [128, 16], f32, kind="ExternalInput")
    selb_c = nc.dram_tensor("selb_c", [128, 16], f32, kind="ExternalInput")

    hself_v = hself[:, :].rearrange("(t p) d -> p t d", p=P)
    nbr_v = nbr[:, :].rearrange("(t p) m -> p t m", p=P)
    out_v = out[:, :].rearrange("(t p) c -> p t c", p=P)

    with tile.TileContext(nc) as tc:
        with (
            tc.tile_pool(name="const", bufs=1) as cst,
            tc.tile_pool(name="gather", bufs=2) as gp,
            tc.tile_pool(name="io", bufs=2) as io,
            tc.tile_pool(name="wk", bufs=2) as sb,
            tc.tile_pool(name="ps", bufs=1, space="PSUM") as ps,
        ):
            idf = cst.tile([128, 128], f32)
            nc.sync.dma_start(out=idf[:], in_=idf_c[:, :])
            idh = cst.tile([128, 128], f16)
            nc.sync.dma_start(out=idh[:], in_=idh_c[:, :])
            sela = cst.tile([128, 16], f32)
            nc.sync.dma_start(out=sela[:], in_=sela_c[:, :])
            selb = cst.tile([128, 16], f32)
            nc.sync.dma_start(out=selb[:], in_=selb_c[:, :])
            wgt = cst.tile([128, 128], f32)
            nc.sync.dma_start(out=wgt[:], in_=wg[:, :])

            for grp in range(NGRP):
                t0 = grp * G

                ht = io.tile([P, G, 128], f32, tag="ht")
                nc.sync.dma_start(out=ht[:], in_=hself_v[:, t0:t0 + G, :])
                idxt = io.tile([P, G, M], i32, tag="idxt")
                nc.sync.dma_start(out=idxt[:], in_=nbr_v[:, t0:t0 + G, :])

                # gather: per (g, m) one per-partition indirect DMA of 512B rows
                nbx = gp.tile([P, G, M, 256], f16, tag="nbx")
                for g in range(G):
                    for m in range(M):
                        nc.gpsimd.indirect_dma_start(
                            out=nbx[:, g, m, :],
                            out_offset=None,
                            in_=htab[:, :],
                            in_offset=IndirectOffsetOnAxis(
                                ap=idxt[:, g, m:m + 1], axis=0),
                        )

                # ---- hc = h * rsqrt(max(sum h^2, 1e-24)) per capsule ----
                sqh = sb.tile([P, G, 128], f32, tag="sqh")
                nc.vector.tensor_tensor(out=sqh[:], in0=ht[:], in1=ht[:], op=AL.mult)
                ss8 = sb.tile([P, G, 8], f32, tag="ss8")
                nc.vector.tensor_reduce(
                    out=ss8[:], in_=sqh[:].rearrange("p g (k e) -> p g k e", e=16),
                    axis=mybir.AxisListType.X, op=AL.add)
                del sqh
                nc.vector.tensor_scalar(out=ss8[:], in0=ss8[:], scalar1=1e-24,
                                        scalar2=None, op0=AL.max)
                ih8 = _rsqrt(nc, sb, "ih8", ss8[:], [P, G, 8])
                hc = sb.tile([P, G, 128], f32, tag="hc")
                nc.vector.tensor_tensor(
                    out=hc[:].rearrange("p g (k e) -> p g k e", e=16),
                    in0=ht[:].rearrange("p g (k e) -> p g k e", e=16),
                    in1=ih8[:].unsqueeze(3).to_broadcast([P, G, 8, 16]),
                    op=AL.mult)
                hch = sb.tile([P, G, 128], f16, tag="hch")
                nc.vector.tensor_copy(out=hch[:], in_=hc[:])

                # ---- PE: nb2 sum (hi|lo), transposes, z = G_bd^T-apply ----
                psA = ps.tile([P, G, 256], f32, tag="psA")
                for g in range(G):
                    for m in range(M):
                        nc.tensor.matmul(out=psA[:, g, :], lhsT=idh[:],
                                         rhs=nbx[:, g, m, :],
                                         start=(m == 0), stop=(m == M - 1))
                nbs = sb.tile([P, G, 128], f32, tag="nbs")
                nc.scalar.copy(out=nbs[:], in_=psA[:, :, 0:128])
                nc.vector.tensor_tensor(out=nbs[:], in0=nbs[:],
                                        in1=psA[:, :, 128:256], op=AL.add)

                psT = ps.tile([P, G, 2, 128], f32, tag="psT")
                for g in range(G):
                    nc.tensor.matmul(out=psT[:, g, 0, :], lhsT=hc[:, g, :],
                                     rhs=idf[:], start=True, stop=True)
                    nc.tensor.matmul(out=psT[:, g, 1, :], lhsT=nbs[:, g, :],
                                     rhs=idf[:], start=True, stop=True)
                hcT = sb.tile([P, G, 128], f32, tag="hcT")
                nc.scalar.copy(out=hcT[:], in_=psT[:, :, 0, :])
                nbsT = sb.tile([P, G, 128], f32, tag="nbsT")
                nc.scalar.copy(out=nbsT[:], in_=psT[:, :, 1, :])

                psZ = ps.tile([P, G, 128], f32, tag="psZ")
                for g in range(G):
                    nc.tensor.matmul(out=psZ[:, g, :], lhsT=wgt[:],
                                     rhs=hcT[:, g, :], start=True, stop=True)

                a1 = sb.tile([P, G, 128], f32, tag="a1")
                nc.vector.tensor_tensor(out=a1[:], in0=hcT[:], in1=psZ[:], op=AL.mult)
                d1 = sb.tile([P, G, 128], f32, tag="d1")
                nc.vector.tensor_tensor(out=d1[:], in0=nbsT[:], in1=psZ[:], op=AL.mult)

                # rows 0..7 = att logits, rows 8..15 = diagv (accumulated)
                psl = ps.tile([16, G, 128], f32, tag="psl")
                for g in range(G):
                    nc.tensor.matmul(out=psl[:, g, :], lhsT=sela[:],
                                     rhs=a1[:, g, :], start=True, stop=False)
                    nc.tensor.matmul(out=psl[:, g, :], lhsT=selb[:],
                                     rhs=d1[:, g, :], start=False, stop=True)
                lg = sb.tile([16, G, 128], f32, tag="lg")
                nc.scalar.copy(out=lg[:], in_=psl[:])
                psLT = ps.tile([P, G, 16], f32, tag="psLT")
                for g in range(G):
                    nc.tensor.matmul(out=psLT[:, g, :], lhsT=lg[:, g, :],
                                     rhs=idf[0:16, 0:16], start=True, stop=True)
                ad = sb.tile([P, G, 16], f32, tag="ad")
                nc.scalar.copy(out=ad[:], in_=psLT[:])
                # attl = ad[:, :, 0:8], diagv = ad[:, :, 8:16]

                # ---- softmax over k ----
                outt = io.tile([P, G, 24], f32, tag="outt")
                att = outt[:, :, 16:24]
                mx = sb.tile([P, G, 1], f32, tag="mx")
                nc.vector.tensor_reduce(out=mx[:], in_=ad[:, :, 0:8],
                                        axis=mybir.AxisListType.X, op=AL.max)
                e8 = sb.tile([P, G, 8], f32, tag="e8")
                nc.vector.tensor_tensor(
                    out=e8[:], in0=ad[:, :, 0:8],
                    in1=mx[:].to_broadcast([P, G, 8]), op=AL.subtract)
                nc.scalar.activation(out=e8[:], in_=e8[:], func=AF.Exp)
                se = sb.tile([P, G, 1], f32, tag="se")
                nc.vector.tensor_reduce(out=se[:], in_=e8[:],
                                        axis=mybir.AxisListType.X, op=AL.add)
                nc.vector.reciprocal(out=se[:], in_=se[:])
                nc.vector.tensor_tensor(out=att, in0=e8[:],
                                        in1=se[:].to_broadcast([P, G, 8]),
                                        op=AL.mult)

                # ---- ssq over 16 hi planes: ACT squares -> PE accumulate ----
                sqf = sb.tile([P, G, M, 128], f16, tag="sqf")
                nc.scalar.activation(out=sqf[:], in_=nbx[:, :, :, 0:128],
                                     func=AF.Square)
                psQ = ps.tile([P, G, 128], f32, tag="psQ")
                for g in range(G):
                    for m in range(M):
                        nc.tensor.matmul(out=psQ[:, g, :], lhsT=idh[:],
                                         rhs=sqf[:, g, m, :],
                                         start=(m == 0), stop=(m == M - 1))

                # ---- median network (fp16, in-place on hi planes) ----
                spare = sb.tile([P, G, 128], f16, tag="spare")
                slots = [nbx[:, :, m, 0:128] for m in range(16)]
                spare_ap = spare[:]
                for (i, j, nmn, nmx) in MEDIAN_NET:
                    if nmn and nmx:
                        nc.vector.tensor_tensor(out=spare_ap, in0=slots[i],
                                                in1=slots[j], op=AL.min)
                        nc.vector.tensor_tensor(out=slots[j], in0=slots[i],
                                                in1=slots[j], op=AL.max)
                        slots[i], spare_ap = spare_ap, slots[i]
                    elif nmx:
                        nc.vector.tensor_tensor(out=slots[j], in0=slots[i],
                                                in1=slots[j], op=AL.max)
                    else:
                        nc.vector.tensor_tensor(out=slots[i], in0=slots[i],
                                                in1=slots[j], op=AL.min)
                med = sb.tile([P, G, 128], f32, tag="med")
                medh = sb.tile([P, G, 128], f16, tag="medh")
                nc.vector.tensor_tensor(out=medh[:], in0=hch[:], in1=slots[7],
                                        op=AL.max)
                nc.vector.tensor_tensor(out=med[:], in0=medh[:], in1=slots[8],
                                        op=AL.min)

                # middle = med * rsqrt(max(ssq + hc^2, 1e-24))
                hsq = sb.tile([P, G, 128], f32, tag="hsq")
                nc.vector.tensor_tensor(out=hsq[:], in0=hc[:], in1=hc[:], op=AL.mult)
                ssq = sb.tile([P, G, 128], f32, tag="ssq")
                nc.vector.tensor_tensor(out=ssq[:], in0=psQ[:], in1=hsq[:], op=AL.add)
                nc.vector.tensor_scalar(out=ssq[:], in0=ssq[:], scalar1=1e-24,
                                        scalar2=None, op0=AL.max)
                u17 = _rsqrt(nc, sb, "u17", ssq[:], [P, G, 128])
                mid = sb.tile([P, G, 128], f32, tag="mid")
                nc.vector.tensor_tensor(out=mid[:], in0=med[:], in1=u17[:], op=AL.mult)

                # ---- c_f = sum_k diagv_k (hc-mid)^2 ----
                y = sb.tile([P, G, 128], f32, tag="y")
                nc.vector.tensor_tensor(out=y[:], in0=hc[:], in1=mid[:], op=AL.subtract)
                yd = sb.tile([P, G, 128], f32, tag="yd")
                nc.vector.tensor_tensor(
                    out=yd[:].rearrange("p g (k e) -> p g k e", e=16),
                    in0=y[:].rearrange("p g (k e) -> p g k e", e=16),
                    in1=ad[:, :, 8:16].unsqueeze(3).to_broadcast([P, G, 8, 16]),
                    op=AL.mult)
                nc.vector.tensor_tensor(out=yd[:], in0=yd[:], in1=y[:], op=AL.mult)
                cc = sb.tile([P, G, 16], f32, tag="cc")
                nc.vector.tensor_reduce(
                    out=cc[:], in_=yd[:].rearrange("p g (k e) -> p g e k", e=16),
                    axis=mybir.AxisListType.X, op=AL.add)

                # ---- flag = any(0 <= c < 1) ----
                fl1 = sb.tile([P, G, 16], f32, tag="fl1")
                nc.vector.tensor_scalar(out=fl1[:], in0=cc[:], scalar1=0.0,
                                        scalar2=None, op0=AL.is_ge)
                fl2 = sb.tile([P, G, 16], f32, tag="fl2")
                nc.vector.scalar_tensor_tensor(
                    out=fl2[:], in0=cc[:], scalar=1.0, in1=fl1[:],
                    op0=AL.is_lt, op1=AL.logical_and)
                flag = sb.tile([P, G, 1], f32, tag="flag")
                nc.vector.tensor_reduce(out=flag[:], in_=fl2[:],
                                        axis=mybir.AxisListType.X, op=AL.max)
                flagbar = sb.tile([P, G, 1], f32, tag="flagbar")
                nc.vector.tensor_scalar(out=flagbar[:], in0=flag[:], scalar1=-1.0,
                                        scalar2=1.0, op0=AL.mult, op1=AL.add)
                feps = sb.tile([P, G, 1], f32, tag="feps")
                nc.vector.tensor_scalar(out=feps[:], in0=flag[:], scalar1=1e-6,
                                        scalar2=None, op0=AL.mult)

                # ---- routing iterations ----
                # C_k = sum_f c_f hc_kf^2 (iteration-invariant; reuse hsq=hc^2)
                nc.vector.tensor_tensor(
                    out=hsq[:].rearrange("p g (k e) -> p g k e", e=16),
                    in0=hsq[:].rearrange("p g (k e) -> p g k e", e=16),
                    in1=cc[:].unsqueeze(2).to_broadcast([P, G, 8, 16]),
                    op=AL.mult)
                Ck = sb.tile([P, G, 8], f32, tag="Ck")
                nc.vector.tensor_reduce(
                    out=Ck[:], in_=hsq[:].rearrange("p g (k e) -> p g k e", e=16),
                    axis=mybir.AxisListType.X, op=AL.add)
                # hw_k f = c_f * hc_kf (for B term)
                hw = sb.tile([P, G, 128], f32, tag="hw")
                nc.vector.tensor_tensor(
                    out=hw[:].rearrange("p g (k e) -> p g k e", e=16),
                    in0=hc[:].rearrange("p g (k e) -> p g k e", e=16),
                    in1=cc[:].unsqueeze(2).to_broadcast([P, G, 8, 16]),
                    op=AL.mult)

                # ix0 = sum_k att_k hc_k
                nh = sb.tile([P, G, 128], f32, tag="nh")
                nc.vector.tensor_tensor(
                    out=nh[:].rearrange("p g (k e) -> p g k e", e=16),
                    in0=hc[:].rearrange("p g (k e) -> p g k e", e=16),
                    in1=att.unsqueeze(3).to_broadcast([P, G, 8, 16]),
                    op=AL.mult)
                ix = sb.tile([P, G, 16], f32, tag="ix")
                nc.vector.tensor_reduce(
                    out=ix[:], in_=nh[:].rearrange("p g (k e) -> p g e k", e=16),
                    axis=mybir.AxisListType.X, op=AL.add)

                for _ in range(iterat):
                    # A = sum_f c_f ix_f^2
                    ixq = sb.tile([P, G, 16], f32, tag="ixq")
                    nc.vector.tensor_tensor(out=ixq[:], in0=ix[:], in1=ix[:],
                                            op=AL.mult)
                    nc.vector.tensor_tensor(out=ixq[:], in0=ixq[:], in1=cc[:],
                                            op=AL.mult)
                    Aa = sb.tile([P, G, 1], f32, tag="Aa")
                    nc.vector.tensor_reduce(out=Aa[:], in_=ixq[:],
                                            axis=mybir.AxisListType.X, op=AL.add)
                    # B_k = sum_f hw_kf ix_f
                    bh = sb.tile([P, G, 128], f32, tag="bh")
                    nc.vector.tensor_tensor(
                        out=bh[:].rearrange("p g (k e) -> p g k e", e=16),
                        in0=hw[:].rearrange("p g (k e) -> p g k e", e=16),
                        in1=ix[:].unsqueeze(2).to_broadcast([P, G, 8, 16]),
                        op=AL.mult)
                    Bk = sb.tile([P, G, 8], f32, tag="Bk")
                    nc.vector.tensor_reduce(
                        out=Bk[:], in_=bh[:].rearrange("p g (k e) -> p g k e", e=16),
                        axis=mybir.AxisListType.X, op=AL.add)
                    # r = A - 2B + C
                    r8 = sb.tile([P, G, 8], f32, tag="r8")
                    nc.vector.scalar_tensor_tensor(
                        out=r8[:], in0=Bk[:], scalar=-2.0, in1=Ck[:],
                        op0=AL.mult, op1=AL.add)
                    nc.vector.tensor_tensor(out=r8[:], in0=r8[:],
                                            in1=Aa[:].to_broadcast([P, G, 8]),
                                            op=AL.add)
                    er = sb.tile([P, G, 8], f32, tag="er")
                    nc.scalar.activation(out=er[:], in_=r8[:], func=AF.Exp,
                                         scale=-0.5)
                    # p = flag ? 1e-6 : er * rsqrt(max(sum er^2,1e-24))
                    es = sb.tile([P, G, 8], f32, tag="es")
                    nc.vector.tensor_tensor(out=es[:], in0=er[:], in1=er[:],
                                            op=AL.mult)
                    s1 = sb.tile([P, G, 1], f32, tag="s1")
                    nc.vector.tensor_reduce(out=s1[:], in_=es[:],
                                            axis=mybir.AxisListType.X, op=AL.add)
                    nc.vector.tensor_scalar(out=s1[:], in0=s1[:], scalar1=1e-24,
                                            scalar2=None, op0=AL.max)
                    u1 = _rsqrt(nc, sb, "u1", s1[:], [P, G, 1])
                    pr = sb.tile([P, G, 8], f32, tag="pr")
                    nc.vector.tensor_tensor(out=pr[:], in0=er[:],
                                            in1=u1[:].to_broadcast([P, G, 8]),
                                            op=AL.mult)
                    nc.vector.tensor_tensor(out=pr[:], in0=pr[:],
                                            in1=flagbar[:].to_broadcast([P, G, 8]),
                                            op=AL.mult)
                    nc.vector.tensor_tensor(out=pr[:], in0=pr[:],
                                            in1=feps[:].to_broadcast([P, G, 8]),
                                            op=AL.add)
                    # ap = att * p ; den = sum ap + 1e-9 ; num = sum_k ap_k hc_k
                    ap8 = sb.tile([P, G, 8], f32, tag="ap8")
                    nc.vector.tensor_tensor(out=ap8[:], in0=att, in1=pr[:],
                                            op=AL.mult)
                    den = sb.tile([P, G, 1], f32, tag="den")
                    nc.vector.tensor_reduce(out=den[:], in_=ap8[:],
                                            axis=mybir.AxisListType.X, op=AL.add)
                    nc.vector.tensor_scalar(out=den[:], in0=den[:], scalar1=1e-9,
                                            scalar2=None, op0=AL.add)
                    nc.vector.reciprocal(out=den[:], in_=den[:])
                    nh2 = sb.tile([P, G, 128], f32, tag="nh2")
                    nc.vector.tensor_tensor(
                        out=nh2[:].rearrange("p g (k e) -> p g k e", e=16),
                        in0=hc[:].rearrange("p g (k e) -> p g k e", e=16),
                        in1=ap8[:].unsqueeze(3).to_broadcast([P, G, 8, 16]),
                        op=AL.mult)
                    nc.vector.tensor_reduce(
                        out=ix[:], in_=nh2[:].rearrange("p g (k e) -> p g e k", e=16),
                        axis=mybir.AxisListType.X, op=AL.add)
                    nc.vector.tensor_tensor(out=ix[:], in0=ix[:],
                                            in1=den[:].to_broadcast([P, G, 16]),
                                            op=AL.mult)

                # ---- x = normalize(ix) ----
                xq = sb.tile([P, G, 16], f32, tag="xq")
                nc.vector.tensor_tensor(out=xq[:], in0=ix[:], in1=ix[:], op=AL.mult)
                s2 = sb.tile([P, G, 1], f32, tag="s2")
                nc.vector.tensor_reduce(out=s2[:], in_=xq[:],
                                        axis=mybir.AxisListType.X, op=AL.add)
                nc.vector.tensor_scalar(out=s2[:], in0=s2[:], scalar1=1e-24,
                                        scalar2=None, op0=AL.max)
                u2 = _rsqrt(nc, sb, "u2", s2[:], [P, G, 1])
                nc.vector.tensor_tensor(out=outt[:, :, 0:16], in0=ix[:],
                                        in1=u2[:].to_broadcast([P, G, 16]),
                                        op=AL.mult)

                nc.gpsimd.dma_start(out=out_v[:, t0:t0 + G, :], in_=outt[:])

                if debug:
                    for nm, src in [("hc", hc[:]), ("nbs", nbs[:]),
                                    ("ad", ad[:]), ("med", med[:]),
                                    ("mid", mid[:]), ("cc", cc[:]),
                                    ("flag", flag[:]), ("ssq", ssq[:]),
                                    ("ix0", ix[:])]:
                        if nm in dbg:
                            nc.gpsimd.dma_start(
                                out=dbg_v[nm][:, t0:t0 + G, :], in_=src)

    nc.compile()
    return nc


_IDF = np.eye(128, dtype=np.float32)
_IDH = np.eye(128, dtype=np.float16)
_SELA = np.zeros((128, 16), dtype=np.float32)
_SELB = np.zeros((128, 16), dtype=np.float32)
for _e in range(128):
    _SELA[_e, _e // 16] = 1.0
    _SELB[_e, 8 + _e // 16] = 1.0

_PROGRAM_CACHE = {}


def _get_program(iterat: int):
    if iterat not in _PROGRAM_CACHE:
        _PROGRAM_CACHE[iterat] = build_program(iterat)
    return _PROGRAM_CACHE[iterat]


def kernel(h, neighbors, query, key_w, iterat, max_iter):
    del max_iter
    h = np.asarray(h, dtype=np.float32)
    neighbors = np.asarray(neighbors).astype(np.int64).reshape(N, M)
    query = np.asarray(query, dtype=np.float32)
    key_w = np.asarray(key_w, dtype=np.float32)
    it = int(iterat)

    # hi|lo fp16 split table: row = [fp16(h) | fp16(h - fp16(h))], 512B/row
    hi = h.astype(np.float16)
    lo = (h - hi.astype(np.float32)).astype(np.float16)
    htab = np.concatenate([hi, lo], axis=1)  # [N, 256] fp16

    # blockdiag((query @ key_w.T).T)
    Gm = (query @ key_w.T).astype(np.float32)
    wg = np.zeros((D, D), dtype=np.float32)
    for k in range(K):
        wg[k * DD:(k + 1) * DD, k * DD:(k + 1) * DD] = Gm.T

    nbr32 = np.clip(neighbors, 0, N - 1).astype(np.int32)

    nc = _get_program(it)
    in_maps = []
    for c in range(NC):
        sl = slice(c * NPC, (c + 1) * NPC)
        in_maps.append({
            "htab": htab,
            "hself": h[sl],
            "nbr": nbr32[sl],
            "wg": wg,
            "idf_c": _IDF,
            "idh_c": _IDH,
            "sela_c": _SELA,
            "selb_c": _SELB,
        })
    res = run_bass_kernel_spmd(nc, in_maps, core_ids=list(range(NC)))
    outs = np.concatenate([r["out"] for r in res.results], axis=0)  # [N, 24]
    x = np.ascontiguousarray(outs[:, 0:16])
    att = np.ascontiguousarray(outs[:, 16:24])
    return x, att


# revision 23
# speedup vs baseline: 1.0091x; 1.0091x over previous
"""Trainium2 Bass kernel for nn_AttentionLayer_9792525435162 (gnn_message_passing).

Math (reference reformulated, validated in fp64/fp32/fp16 numpy sims):
  hc    = normalize(h.reshape(N,8,16), axis=2)
  att   = softmax_k( <hc_k, G hc_k> )          with G = query @ key_w.T
  diagv = <nb_sum_k, G hc_k>                    with nb_sum = sum_m h[neighbors]
  med17 = clamp(hc, S7, S8)  where S7,S8 = central pair of the 16 gathered
          neighbor values per (node, feature)   [median-of-17 identity]
  middle= med17 / max(||sub||_2 over 17, 1e-12)
  c_f   = sum_k diagv_k (hc_kf - middle_kf)^2   (cov, k-independent)
  flag  = any_f(0 <= c_f < 1)                   (dets==NaN path in reference)
  3x routing iterations with p = flag ? 1e-6 : right/max(||right||,1e-12),
  right = exp(-0.5 sum_f c_f (ix_f - hc_kf)^2); x = normalize(ix).

Sharding: data-parallel over nodes, 8192 nodes/core on 8 cores. The h table
is replicated per-core (hi|lo fp16 split rows, 512B each) and gathered with
per-partition indirect DMAs. All rsqrts via exp(-0.5 ln(max(s,eps))) + one
Newton step so ACT stays on the natural_log_exp_and_others table set.
"""

import sys

if "/opt/trn_rl_repo" not in sys.path:
    sys.path.insert(0, "/opt/trn_rl_repo")

import numpy as np

import concourse.bacc as bacc
import concourse.tile as tile
from concourse import mybir
from concourse.bass import IndirectOffsetOnAxis
from concourse.bass_utils import run_bass_kernel_spmd

f32 = mybir.dt.float32
f16 = mybir.dt.float16
i32 = mybir.dt.int32

N, M, D, K, DD = 65536, 16, 128, 8, 16
NC = 8
NPC = N // NC          # nodes per core
P = 128
NT = NPC // P          # tiles per core (64)
G = 4                  # tiles per group
NGRP = NT // G

AL = mybir.AluOpType
AF = mybir.ActivationFunctionType

# Median-of-16 central-pair selection network (pruned Batcher odd-even
# mergesort; 53 CEs, 92 ops). (i, j, need_min, need_max).
MEDIAN_NET = [
    (0, 1, 1, 1), (2, 3, 1, 1), (0, 2, 1, 1), (1, 3, 1, 1), (1, 2, 1, 1),
    (4, 5, 1, 1), (6, 7, 1, 1), (4, 6, 1, 1), (5, 7, 1, 1), (5, 6, 1, 1),
    (0, 4, 1, 1), (2, 6, 1, 1), (2, 4, 1, 1), (1, 5, 1, 1), (3, 7, 1, 1),
    (3, 5, 1, 1), (1, 2, 1, 1), (3, 4, 1, 1), (5, 6, 1, 1), (8, 9, 1, 1),
    (10, 11, 1, 1), (8, 10, 1, 1), (9, 11, 1, 1), (9, 10, 1, 1),
    (12, 13, 1, 1), (14, 15, 1, 1), (12, 14, 1, 1), (13, 15, 1, 1),
    (13, 14, 1, 1), (8, 12, 1, 1), (10, 14, 1, 1), (10, 12, 1, 1),
    (9, 13, 1, 1), (11, 15, 1, 1), (11, 13, 1, 1), (9, 10, 1, 1),
    (11, 12, 1, 1), (13, 14, 1, 1), (0, 8, 0, 1), (4, 12, 1, 0),
    (4, 8, 0, 1), (2, 10, 0, 1), (6, 14, 1, 0), (6, 10, 1, 0), (6, 8, 0, 1),
    (1, 9, 0, 1), (5, 13, 1, 0), (5, 9, 0, 1), (3, 11, 0, 1), (7, 15, 1, 0),
    (7, 11, 1, 0), (7, 9, 1, 0), (7, 8, 1, 1),
]


def _rsqrt(nc, sb, tag, src_ap, shape, newton_src=None):
    """out = 1/sqrt(src) via exp(-0.5*ln(src)) + one Newton step.

    src must already be clamped positive. newton_src: AP of the clamped s
    (defaults to src_ap). Returns the refined [*, ...] f32 tile.
    """
    ln_t = sb.tile(shape, f32, tag=tag + "_ln")
    nc.scalar.activation(out=ln_t[:], in_=src_ap, func=AF.Ln)
    y0 = sb.tile(shape, f32, tag=tag + "_y0")
    nc.scalar.activation(out=y0[:], in_=ln_t[:], func=AF.Exp, scale=-0.5)
    if newton_src is None:
        newton_src = src_ap
    # y1 = y0 * (1.5 - 0.5*s*y0^2)
    t1 = sb.tile(shape, f32, tag=tag + "_t1")
    nc.vector.tensor_tensor(out=t1[:], in0=y0[:], in1=y0[:], op=AL.mult)
    nc.vector.tensor_tensor(out=t1[:], in0=t1[:], in1=newton_src, op=AL.mult)
    nc.vector.tensor_scalar(out=t1[:], in0=t1[:], scalar1=-0.5, scalar2=1.5,
                            op0=AL.mult, op1=AL.add)
    nc.vector.tensor_tensor(out=t1[:], in0=t1[:], in1=y0[:], op=AL.mult)
    return t1


def build_program(iterat: int, debug=False):
    nc = bacc.Bacc("TRN2", target_bir_lowering=False)

    htab = nc.dram_tensor("htab", [N, 256], f16, kind="ExternalInput")
    hself = nc.dram_tensor("hself", [NPC, D], f32, kind="ExternalInput")
    nbr = nc.dram_tensor("nbr", [NPC, M], i32, kind="ExternalInput")
    wg = nc.dram_tensor("wg", [D, D], f32, kind="ExternalInput")  # blockdiag(G.T)
    out = nc.dram_tensor("out", [NPC, 24], f32, kind="ExternalOutput")
    dbg = {}
    _dbg_all = [("hc", 128), ("nbs", 128), ("ad", 16), ("med", 128),
                ("mid", 128), ("cc", 16), ("flag", 1), ("ssq", 128),
                ("ix0", 16)]
    if debug:
        sel_dbg = [x for x in _dbg_all
                   if debug is True or x[0] in debug]
        for nm, w in sel_dbg:
            dbg[nm] = nc.dram_tensor("dbg_" + nm, [NPC, w], f32,
                                     kind="ExternalOutput")
        dbg_v = {nm: t[:, :].rearrange("(t p) c -> p t c", p=P)
                 for nm, t in dbg.items()}

    idf_c = nc.dram_tensor("idf_c", [128, 128], f32, kind="ExternalInput")
    idh_c = nc.dram_tensor("idh_c", [128, 128], f16, kind="ExternalInput")
    sela_c = nc.dram_tensor("sela_c", [128, 16], f32, kind="ExternalInput")
    selb_c = nc.dram_tensor("selb_c", [128, 16], f32, kind="ExternalInput")

    hself_v = hself[:, :].rearrange("(t p) d -> p t d", p=P)
    nbr_v = nbr[:, :].rearrange("(t p) m -> p t m", p=P)
    out_v = out[:, :].rearrange("(t p) c -> p t c", p=P)

    with tile.TileContext(nc) as tc:
        with (
            tc.tile_pool(name="const", bufs=1) as cst,
            tc.tile_pool(name="gather", bufs=2) as gp,
            tc.tile_pool(name="io", bufs=2) as io,
            tc.tile_pool(name="wk", bufs=2) as sb,
            tc.tile_pool(name="ps", bufs=1, space="PSUM") as ps,
        ):
            idf = cst.tile([128, 128], f32)
            nc.sync.dma_start(out=idf[:], in_=idf_c[:, :])
            idh = cst.tile([128, 128], f16)
            nc.sync.dma_start(out=idh[:], in_=idh_c[:, :])
            sela = cst.tile([128, 16], f32)
            nc.sync.dma_start(out=sela[:], in_=sela_c[:, :])
            selb = cst.tile([128, 16], f32)
            nc.sync.dma_start(out=selb[:], in_=selb_c[:, :])
            wgt = cst.tile([128, 128], f32)
            nc.sync.dma_start(out=wgt[:], in_=wg[:, :])

            for grp in range(NGRP):
                t0 = grp * G

                ht = io.tile([P, G, 128], f32, tag="ht")
                nc.sync.dma_start(out=ht[:], in_=hself_v[:, t0:t0 + G, :])
                idxt = io.tile([P, G, M], i32, tag="idxt")
                nc.sync.dma_start(out=idxt[:], in_=nbr_v[:, t0:t0 + G, :])

                # gather: per (g, m) one per-partition indirect DMA of 512B rows
                nbx = gp.tile([P, G, M, 256], f16, tag="nbx")
                for g in range(G):
                    for m in range(M):
                        nc.gpsimd.indirect_dma_start(
                            out=nbx[:, g, m, :],
                            out_offset=None,
                            in_=htab[:, :],
                            in_offset=IndirectOffsetOnAxis(
                                ap=idxt[:, g, m:m + 1], axis=0),
                        )

                # ---- hc = h * rsqrt(max(sum h^2, 1e-24)) per capsule ----
                sqh = sb.tile([P, G, 128], f32, tag="sqh")
                nc.vector.tensor_tensor(out=sqh[:], in0=ht[:], in1=ht[:], op=AL.mult)
                ss8 = sb.tile([P, G, 8], f32, tag="ss8")
                nc.vector.tensor_reduce(
                    out=ss8[:], in_=sqh[:].rearrange("p g (k e) -> p g k e", e=16),
                    axis=mybir.AxisListType.X, op=AL.add)
                del sqh
                nc.vector.tensor_scalar(out=ss8[:], in0=ss8[:], scalar1=1e-24,
                                        scalar2=None, op0=AL.max)
                ih8 = _rsqrt(nc, sb, "ih8", ss8[:], [P, G, 8])
                hc = sb.tile([P, G, 128], f32, tag="hc")
                nc.vector.tensor_tensor(
                    out=hc[:].rearrange("p g (k e) -> p g k e", e=16),
                    in0=ht[:].rearrange("p g (k e) -> p g k e", e=16),
                    in1=ih8[:].unsqueeze(3).to_broadcast([P, G, 8, 16]),
                    op=AL.mult)
                hch = sb.tile([P, G, 128], f16, tag="hch")
                nc.vector.tensor_copy(out=hch[:], in_=hc[:])

                # ---- PE: nb2 sum (hi|lo), transposes, z = G_bd^T-apply ----
                psA = ps.tile([P, G, 128], f32, tag="psA")
                for g in range(G):
                    for m in range(M):
                        nc.tensor.matmul(out=psA[:, g, :], lhsT=idh[:],
                                         rhs=nbx[:, g, m, 0:128],
                                         start=(m == 0), stop=False)
                    for m in range(M):
                        nc.tensor.matmul(out=psA[:, g, :], lhsT=idh[:],
                                         rhs=nbx[:, g, m, 128:256],
                                         start=False, stop=(m == M - 1))
                nbs = sb.tile([P, G, 128], f32, tag="nbs")
                nc.scalar.copy(out=nbs[:], in_=psA[:])

                psT = ps.tile([P, G, 2, 128], f32, tag="psT")
                for g in range(G):
                    nc.tensor.matmul(out=psT[:, g, 0, :], lhsT=hc[:, g, :],
                                     rhs=idf[:], start=True, stop=True)
                    nc.tensor.matmul(out=psT[:, g, 1, :], lhsT=nbs[:, g, :],
                                     rhs=idf[:], start=True, stop=True)
                hcT = sb.tile([P, G, 128], f32, tag="hcT")
                nc.scalar.copy(out=hcT[:], in_=psT[:, :, 0, :])
                nbsT = sb.tile([P, G, 128], f32, tag="nbsT")
                nc.scalar.copy(out=nbsT[:], in_=psT[:, :, 1, :])

                psZ = ps.tile([P, G, 128], f32, tag="psZ")
                for g in range(G):
                    nc.tensor.matmul(out=psZ[:, g, :], lhsT=wgt[:],
                                     rhs=hcT[:, g, :], start=True, stop=True)

                a1 = sb.tile([P, G, 128], f32, tag="a1")
                nc.vector.tensor_tensor(out=a1[:], in0=hcT[:], in1=psZ[:], op=AL.mult)
                d1 = sb.tile([P, G, 128], f32, tag="d1")
                nc.vector.tensor_tensor(out=d1[:], in0=nbsT[:], in1=psZ[:], op=AL.mult)

                # rows 0..7 = att logits, rows 8..15 = diagv (accumulated)
                psl = ps.tile([16, G, 128], f32, tag="psl")
                for g in range(G):
                    nc.tensor.matmul(out=psl[:, g, :], lhsT=sela[:],
                                     rhs=a1[:, g, :], start=True, stop=False)
                    nc.tensor.matmul(out=psl[:, g, :], lhsT=selb[:],
                                     rhs=d1[:, g, :], start=False, stop=True)
                lg = sb.tile([16, G, 128], f32, tag="lg")
                nc.scalar.copy(out=lg[:], in_=psl[:])
                psLT = ps.tile([P, G, 16], f32, tag="psLT")
                for g in range(G):
                    nc.tensor.matmul(out=psLT[:, g, :], lhsT=lg[:, g, :],
                                     rhs=idf[0:16, 0:16], start=True, stop=True)
                ad = sb.tile([P, G, 16], f32, tag="ad")
                nc.scalar.copy(out=ad[:], in_=psLT[:])
                # attl = ad[:, :, 0:8], diagv = ad[:, :, 8:16]

                # ---- softmax over k ----
                outt = io.tile([P, G, 24], f32, tag="outt")
                att = outt[:, :, 16:24]
                mx = sb.tile([P, G, 1], f32, tag="mx")
                nc.vector.tensor_reduce(out=mx[:], in_=ad[:, :, 0:8],
                                        axis=mybir.AxisListType.X, op=AL.max)
                e8 = sb.tile([P, G, 8], f32, tag="e8")
                nc.vector.tensor_tensor(
                    out=e8[:], in0=ad[:, :, 0:8],
                    in1=mx[:].to_broadcast([P, G, 8]), op=AL.subtract)
                nc.scalar.activation(out=e8[:], in_=e8[:], func=AF.Exp)
                se = sb.tile([P, G, 1], f32, tag="se")
                nc.vector.tensor_reduce(out=se[:], in_=e8[:],
                                        axis=mybir.AxisListType.X, op=AL.add)
                nc.vector.reciprocal(out=se[:], in_=se[:])
                nc.vector.tensor_tensor(out=att, in0=e8[:],
                                        in1=se[:].to_broadcast([P, G, 8]),
                                        op=AL.mult)

                # ---- ssq over 16 hi planes: ACT squares -> PE accumulate ----
                sqf = sb.tile([P, G, M, 128], f16, tag="sqf")
                nc.scalar.activation(out=sqf[:], in_=nbx[:, :, :, 0:128],
                                     func=AF.Square)
                psQ = ps.tile([P, G, 128], f32, tag="psQ")
                for g in range(G):
                    for m in range(M):
                        nc.tensor.matmul(out=psQ[:, g, :], lhsT=idh[:],
                                         rhs=sqf[:, g, m, :],
                                         start=(m == 0), stop=(m == M - 1))

                # ---- median network (fp16, in-place on hi planes) ----
                spare = sb.tile([P, G, 128], f16, tag="spare")
                slots = [nbx[:, :, m, 0:128] for m in range(16)]
                spare_ap = spare[:]
                for (i, j, nmn, nmx) in MEDIAN_NET:
                    if nmn and nmx:
                        nc.vector.tensor_tensor(out=spare_ap, in0=slots[i],
                                                in1=slots[j], op=AL.min)
                        nc.vector.tensor_tensor(out=slots[j], in0=slots[i],
                                                in1=slots[j], op=AL.max)
                        slots[i], spare_ap = spare_ap, slots[i]
                    elif nmx:
                        nc.vector.tensor_tensor(out=slots[j], in0=slots[i],
                                                in1=slots[j], op=AL.max)
                    else:
                        nc.vector.tensor_tensor(out=slots[i], in0=slots[i],
                                                in1=slots[j], op=AL.min)
                med = sb.tile([P, G, 128], f32, tag="med")
                medh = sb.tile([P, G, 128], f16, tag="medh")
                nc.vector.tensor_tensor(out=medh[:], in0=hch[:], in1=slots[7],
                                        op=AL.max)
                nc.vector.tensor_tensor(out=med[:], in0=medh[:], in1=slots[8],
                                        op=AL.min)

                # middle = med * rsqrt(max(ssq + hc^2, 1e-24))
                hsq = sb.tile([P, G, 128], f32, tag="hsq")
                nc.vector.tensor_tensor(out=hsq[:], in0=hc[:], in1=hc[:], op=AL.mult)
                ssq = sb.tile([P, G, 128], f32, tag="ssq")
                nc.vector.tensor_tensor(out=ssq[:], in0=psQ[:], in1=hsq[:], op=AL.add)
                nc.vector.tensor_scalar(out=ssq[:], in0=ssq[:], scalar1=1e-24,
                                        scalar2=None, op0=AL.max)
                u17 = _rsqrt(nc, sb, "u17", ssq[:], [P, G, 128])
                mid = sb.tile([P, G, 128], f32, tag="mid")
                nc.vector.tensor_tensor(out=mid[:], in0=med[:], in1=u17[:], op=AL.mult)

                # ---- c_f = sum_k diagv_k (hc-mid)^2 ----
                y = sb.tile([P, G, 128], f32, tag="y")
                nc.vector.tensor_tensor(out=y[:], in0=hc[:], in1=mid[:], op=AL.subtract)
                yd = sb.tile([P, G, 128], f32, tag="yd")
                nc.vector.tensor_tensor(
                    out=yd[:].rearrange("p g (k e) -> p g k e", e=16),
                    in0=y[:].rearrange("p g (k e) -> p g k e", e=16),
                    in1=ad[:, :, 8:16].unsqueeze(3).to_broadcast([P, G, 8, 16]),
                    op=AL.mult)
                nc.vector.tensor_tensor(out=yd[:], in0=yd[:], in1=y[:], op=AL.mult)
                cc = sb.tile([P, G, 16], f32, tag="cc")
                nc.vector.tensor_reduce(
                    out=cc[:], in_=yd[:].rearrange("p g (k e) -> p g e k", e=16),
                    axis=mybir.AxisListType.X, op=AL.add)

                # ---- flag = any(0 <= c < 1) ----
                fl1 = sb.tile([P, G, 16], f32, tag="fl1")
                nc.vector.tensor_scalar(out=fl1[:], in0=cc[:], scalar1=0.0,
                                        scalar2=None, op0=AL.is_ge)
                fl2 = sb.tile([P, G, 16], f32, tag="fl2")
                nc.vector.scalar_tensor_tensor(
                    out=fl2[:], in0=cc[:], scalar=1.0, in1=fl1[:],
                    op0=AL.is_lt, op1=AL.logical_and)
                flag = sb.tile([P, G, 1], f32, tag="flag")
                nc.vector.tensor_reduce(out=flag[:], in_=fl2[:],
                                        axis=mybir.AxisListType.X, op=AL.max)
                flagbar = sb.tile([P, G, 1], f32, tag="flagbar")
                nc.vector.tensor_scalar(out=flagbar[:], in0=flag[:], scalar1=-1.0,
                                        scalar2=1.0, op0=AL.mult, op1=AL.add)
                feps = sb.tile([P, G, 1], f32, tag="feps")
                nc.vector.tensor_scalar(out=feps[:], in0=flag[:], scalar1=1e-6,
                                        scalar2=None, op0=AL.mult)

                # ---- routing iterations ----
                # C_k = sum_f c_f hc_kf^2 (iteration-invariant; reuse hsq=hc^2)
                nc.vector.tensor_tensor(
                    out=hsq[:].rearrange("p g (k e) -> p g k e", e=16),
                    in0=hsq[:].rearrange("p g (k e) -> p g k e", e=16),
                    in1=cc[:].unsqueeze(2).to_broadcast([P, G, 8, 16]),
                    op=AL.mult)
                Ck = sb.tile([P, G, 8], f32, tag="Ck")
                nc.vector.tensor_reduce(
                    out=Ck[:], in_=hsq[:].rearrange("p g (k e) -> p g k e", e=16),
                    axis=mybir.AxisListType.X, op=AL.add)
                # hw_k f = c_f * hc_kf (for B term)
                hw = sb.tile([P, G, 128], f32, tag="hw")
                nc.vector.tensor_tensor(
                    out=hw[:].rearrange("p g (k e) -> p g k e", e=16),
                    in0=hc[:].rearrange("p g (k e) -> p g k e", e=16),
                    in1=cc[:].unsqueeze(2).to_broadcast([P, G, 8, 16]),
                    op=AL.mult)

                # ix0 = sum_k att_k hc_k
                nh = sb.tile([P, G, 128], f32, tag="nh")
                nc.vector.tensor_tensor(
                    out=nh[:].rearrange("p g (k e) -> p g k e", e=16),
                    in0=hc[:].rearrange("p g (k e) -> p g k e", e=16),
                    in1=att.unsqueeze(3).to_broadcast([P, G, 8, 16]),
                    op=AL.mult)
                ix = sb.tile([P, G, 16], f32, tag="ix")
                nc.vector.tensor_reduce(
                    out=ix[:], in_=nh[:].rearrange("p g (k e) -> p g e k", e=16),
                    axis=mybir.AxisListType.X, op=AL.add)

                for _ in range(iterat):
                    # A = sum_f c_f ix_f^2
                    ixq = sb.tile([P, G, 16], f32, tag="ixq")
                    nc.vector.tensor_tensor(out=ixq[:], in0=ix[:], in1=ix[:],
                                            op=AL.mult)
                    nc.vector.tensor_tensor(out=ixq[:], in0=ixq[:], in1=cc[:],
                                            op=AL.mult)
                    Aa = sb.tile([P, G, 1], f32, tag="Aa")
                    nc.vector.tensor_reduce(out=Aa[:], in_=ixq[:],
                                            axis=mybir.AxisListType.X, op=AL.add)
                    # B_k = sum_f hw_kf ix_f
                    bh = sb.tile([P, G, 128], f32, tag="bh")
                    nc.vector.tensor_tensor(
                        out=bh[:].rearrange("p g (k e) -> p g k e", e=16),
                        in0=hw[:].rearrange("p g (k e) -> p g k e", e=16),
                        in1=ix[:].unsqueeze(2).to_broadcast([P, G, 8, 16]),
                        op=AL.mult)
                    Bk = sb.tile([P, G, 8], f32, tag="Bk")
                    nc.vector.tensor_reduce(
                        out=Bk[:], in_=bh[:].rearrange("p g (k e) -> p g k e", e=16),
                        axis=mybir.AxisListType.X, op=AL.add)
                    # r = A - 2B + C
                    r8 = sb.tile([P, G, 8], f32, tag="r8")
                    nc.vector.scalar_tensor_tensor(
                        out=r8[:], in0=Bk[:], scalar=-2.0, in1=Ck[:],
                        op0=AL.mult, op1=AL.add)
                    nc.vector.tensor_tensor(out=r8[:], in0=r8[:],
                                            in1=Aa[:].to_broadcast([P, G, 8]),
                                            op=AL.add)
                    er = sb.tile([P, G, 8], f32, tag="er")
                    nc.scalar.activation(out=er[:], in_=r8[:], func=AF.Exp,
                                         scale=-0.5)
                    # p = flag ? 1e-6 : er * rsqrt(max(sum er^2,1e-24))
                    es = sb.tile([P, G, 8], f32, tag="es")
                    nc.vector.tensor_tensor(out=es[:], in0=er[:], in1=er[:],
                                            op=AL.mult)
                    s1 = sb.tile([P, G, 1], f32, tag="s1")
                    nc.vector.tensor_reduce(out=s1[:], in_=es[:],
                                            axis=mybir.AxisListType.X, op=AL.add)
                    nc.vector.tensor_scalar(out=s1[:], in0=s1[:], scalar1=1e-24,
                                            scalar2=None, op0=AL.max)
                    u1 = _rsqrt(nc, sb, "u1", s1[:], [P, G, 1])
                    pr = sb.tile([P, G, 8], f32, tag="pr")
                    nc.vector.tensor_tensor(out=pr[:], in0=er[:],
                                            in1=u1[:].to_broadcast([P, G, 8]),
                                            op=AL.mult)
                    nc.vector.tensor_tensor(out=pr[:], in0=pr[:],
                                            in1=flagbar[:].to_broadcast([P, G, 8]),
                                            op=AL.mult)
                    nc.vector.tensor_tensor(out=pr[:], in0=pr[:],
                                            in1=feps[:].to_broadcast([P, G, 8]),
                                            op=AL.add)
                    # ap = att * p ; den = sum ap + 1e-9 ; num = sum_k ap_k hc_k
                    ap8 = sb.tile([P, G, 8], f32, tag="ap8")
                    nc.vector.tensor_tensor(out=ap8[:], in0=att, in1=pr[:],
                                            op=AL.mult)
                    den = sb.tile([P, G, 1], f32, tag="den")
                    nc.vector.tensor_reduce(out=den[:], in_=ap8[:],
                                            axis=mybir.AxisListType.X, op=AL.add)
                    nc.vector.tensor_scalar(out=den[:], in0=den[:], scalar1=1e-9,
                                            scalar2=None, op0=AL.add)
                    nc.vector.reciprocal(out=den[:], in_=den[:])
                    nh2 = sb.tile([P, G, 128], f32, tag="nh2")
                    nc.vector.tensor_tensor(
                        out=nh2[:].rearrange("p g (k e) -> p g k e", e=16),
                        in0=hc[:].rearrange("p g (k e) -> p g k e", e=16),
                        in1=ap8[:].unsqueeze(3).to_broadcast([P, G, 8, 16]),
                        op=AL.mult)
                    nc.vector.tensor_reduce(
                        out=ix[:], in_=nh2[:].rearrange("p g (k e) -> p g e k", e=16),
                        axis=mybir.AxisListType.X, op=AL.add)
                    nc.vector.tensor_tensor(out=ix[:], in0=ix[:],
                                            in1=den[:].to_broadcast([P, G, 16]),
                                            op=AL.mult)

                # ---- x = normalize(ix) ----
                xq = sb.tile([P, G, 16], f32, tag="xq")
                nc.vector.tensor_tensor(out=xq[:], in0=ix[:], in1=ix[:], op=AL.mult)
                s2 = sb.tile([P, G, 1], f32, tag="s2")
                nc.vector.tensor_reduce(out=s2[:], in_=xq[:],
                                        axis=mybir.AxisListType.X, op=AL.add)
                nc.vector.tensor_scalar(out=s2[:], in0=s2[:], scalar1=1e-24,
                                        scalar2=None, op0=AL.max)
                u2 = _rsqrt(nc, sb, "u2", s2[:], [P, G, 1])
                nc.vector.tensor_tensor(out=outt[:, :, 0:16], in0=ix[:],
                                        in1=u2[:].to_broadcast([P, G, 16]),
                                        op=AL.mult)

                nc.gpsimd.dma_start(out=out_v[:, t0:t0 + G, :], in_=outt[:])

                if debug:
                    for nm, src in [("hc", hc[:]), ("nbs", nbs[:]),
                                    ("ad", ad[:]), ("med", med[:]),
                                    ("mid", mid[:]), ("cc", cc[:]),
                                    ("flag", flag[:]), ("ssq", ssq[:]),
                                    ("ix0", ix[:])]:
                        if nm in dbg:
                            nc.gpsimd.dma_start(
                                out=dbg_v[nm][:, t0:t0 + G, :], in_=src)

    nc.compile()
    return nc


_IDF = np.eye(128, dtype=np.float32)
_IDH = np.eye(128, dtype=np.float16)
_SELA = np.zeros((128, 16), dtype=np.float32)
_SELB = np.zeros((128, 16), dtype=np.float32)
for _e in range(128):
    _SELA[_e, _e // 16] = 1.0
    _SELB[_e, 8 + _e // 16] = 1.0

_PROGRAM_CACHE = {}


def _get_program(iterat: int):
    if iterat not in _PROGRAM_CACHE:
        _PROGRAM_CACHE[iterat] = build_program(iterat)
    return _PROGRAM_CACHE[iterat]


def kernel(h, neighbors, query, key_w, iterat, max_iter):
    del max_iter
    h = np.asarray(h, dtype=np.float32)
    neighbors = np.asarray(neighbors).astype(np.int64).reshape(N, M)
    query = np.asarray(query, dtype=np.float32)
    key_w = np.asarray(key_w, dtype=np.float32)
    it = int(iterat)

    # hi|lo fp16 split table: row = [fp16(h) | fp16(h - fp16(h))], 512B/row
    hi = h.astype(np.float16)
    lo = (h - hi.astype(np.float32)).astype(np.float16)
    htab = np.concatenate([hi, lo], axis=1)  # [N, 256] fp16

    # blockdiag((query @ key_w.T).T)
    Gm = (query @ key_w.T).astype(np.float32)
    wg = np.zeros((D, D), dtype=np.float32)
    for k in range(K):
        wg[k * DD:(k + 1) * DD, k * DD:(k + 1) * DD] = Gm.T

    nbr32 = np.clip(neighbors, 0, N - 1).astype(np.int32)

    nc = _get_program(it)
    in_maps = []
    for c in range(NC):
        sl = slice(c * NPC, (c + 1) * NPC)
        in_maps.append({
            "htab": htab,
            "hself": h[sl],
            "nbr": nbr32[sl],
            "wg": wg,
            "idf_c": _IDF,
            "idh_c": _IDH,
            "sela_c": _SELA,
            "selb_c": _SELB,
        })
    res = run_bass_kernel_spmd(nc, in_maps, core_ids=list(range(NC)))
    outs = np.concatenate([r["out"] for r in res.results], axis=0)  # [N, 24]
    x = np.ascontiguousarray(outs[:, 0:16])
    att = np.ascontiguousarray(outs[:, 16:24])
    return x, att
